# revision 29
# baseline (speedup 1.0000x reference)
"""Causal self-attention (QK-RMSNorm + RoPE) on 8 Trainium2 NeuronCores.

Problem: x[2,2048,2048], Wq/Wk/Wv/Wo [2048,2048], 16 heads, head_dim 128.

Sharding: core c handles batch b=c//4 and head group g=c%4 (4 heads,
model cols [512g:512g+512)).  No collectives: each core computes a
PARTIAL o_proj (contraction over its 4 local head dims only) and
returns zT_partial[2048, 2048]; the host sums the 4 partials per batch
and transposes.  This removes the AllGather tail (17-39us per 512KB
gather, serialized) entirely.

Phase A: Q/K/V projections in ONE pass over host-pre-transposed,
host-pretiled xT (contiguous 64KB DMA blocks; contraction dim on
partitions).  Startup DMA is bandwidth-critical: every tensor's slices
are round-robined across the sync/gpsimd/scalar queues in global
consumption order (a single queue is capped at ~1/3 of the per-core
HBM share), wo is deferred to xt chunk 6 (only needed ~250us in), and
cos/sin are host-pretiled to [128,16,64] (no SWDGE).  Both ib-halves
of each xt chunk are projected interleaved per ct so each weight slice
feeds two matmuls.  12 warm-up matmuls on the identity run during the
DMA-dead first ~8us so the HAM clock gate is warm when the first real
matmul issues.  q/k get RMS-norm + RoPE row-wise, are transposed per
head on the PE, and stay resident in SBUF (no DRAM round trip).

Phase B: attention with transposed scores (eT = exp(scale *
kT_blk.T @ qT_chunk)), so the AV matmul (lhsT=v, rhs=eT) directly
yields the transposed output yT[d, i].  Each key block only streams
its causally valid query window (128-granular); a single 128x128
triangular tile masks the diagonal sub-blocks.  Scores are emitted 3
blocks ahead of their exp/AV/den consumers.  The softmax denominator
matmul uses a full ones[128,128] stationary operand so the column
sums come out replicated across all partitions; the drain is then
reciprocal_approx_fast + one multiply on the vector engine.

After each i-chunk's 4 heads are normalized (ytn in SBUF), the
partial o_proj for that chunk runs immediately (64 matmuls:
16 o-tiles x 4 local head contractions) and its z tiles stream out to
DRAM -- no inter-core dependency, so the PE never waits on a
collective.
"""

import math
from contextlib import ExitStack

import numpy as np

import concourse.bass as bass
import concourse.bacc as bacc
import concourse.tile as tile
from concourse import mybir
from concourse.bass_utils import run_bass_kernel_spmd
from concourse.masks import make_identity

P = 128
D = 2048
S = 2048
HD = 128              # head dim
NHL = 4               # heads per core
GW = NHL * HD         # 512, per-core width of head group
CT = D // P           # 16 contraction tiles
ICH = 4               # i-chunks of 512 positions
NCORES = 8
F32 = mybir.dt.float32
F16 = mybir.dt.float16
SCALE = 1.0 / math.sqrt(HD)
EPS = 1.1920928955078125e-07

_program_cache = {}


def build_program():
    if "nc" in _program_cache:
        return _program_cache["nc"]

    nc = bacc.Bacc("TRN2", target_bir_lowering=False, debug=False, num_devices=NCORES)

    # host-pretiled xT: [ica, ct, p, f] = xT[ct*128+p, ica*256+f] so every
    # (chunk, ct) DMA slice is a contiguous 64KB block (strided 512B-segment
    # reads starved the PE during startup)
    xt_in = nc.dram_tensor("xt", [8, CT, P, 256], F16, kind="ExternalInput")
    wq_in = nc.dram_tensor("wq", [D, GW], F16, kind="ExternalInput")
    wk_in = nc.dram_tensor("wk", [D, GW], F16, kind="ExternalInput")
    wv_in = nc.dram_tensor("wv", [D, GW], F16, kind="ExternalInput")
    # Wo.T row slice [512, 2048]: local head dims x full model width
    wo_in = nc.dram_tensor("wo", [GW, D], F16, kind="ExternalInput")
    # host-pretiled rope tables: [p, a, f] = table[a*128+p, f]
    cos_in = nc.dram_tensor("cos", [P, CT, HD // 2], F16, kind="ExternalInput")
    sin_in = nc.dram_tensor("sin", [P, CT, HD // 2], F16, kind="ExternalInput")
    # partial o_proj output, tiled [oc, ic, p, f] = z[oc*128+p, ic*512+f]
    # (contiguous 128KB writes); host sums over 4 cores and untiles
    z_out = nc.dram_tensor("z_out", [CT, ICH, P, 512], F16, kind="ExternalOutput")

    with tile.TileContext(nc) as tc:
        with ExitStack() as ctx:
            const = ctx.enter_context(tc.tile_pool(name="const", bufs=1))

            ident = const.tile([P, P], F16, name="ident")
            make_identity(nc, ident)
            eps_t = const.tile([P, 1], F32, name="eps_t")
            nc.vector.memset(eps_t[:], EPS)
            neg1_t = const.tile([P, 1], F32, name="neg1_t")
            nc.vector.memset(neg1_t[:], -1.0)
            ones_f = const.tile([P, P], F32, name="ones_f")
            nc.vector.memset(ones_f[:], 1.0)
            # full-width ones: den matmul replicates the column sums across
            # all 128 output partitions (same stream cost as 2 partitions)
            ones16 = const.tile([P, P], F16, name="ones16")
            nc.scalar.copy(ones16[:], ones_f[:])
            # tri_m[j, i] = 1 where i >= j (valid), else 0 — diagonal block mask
            tri_m = const.tile([P, P], F16, name="tri_m")
            nc.gpsimd.memset(tri_m[:], 1.0)
            nc.gpsimd.affine_select(
                out=tri_m[:], in_=tri_m[:],
                compare_op=mybir.AluOpType.is_ge,
                fill=0.0,
                base=0,
                pattern=[[1, P]],
                channel_multiplier=-1,
            )

            cos_sb = const.tile([P, CT, HD // 2], F16, name="cos_sb")
            sin_sb = const.tile([P, CT, HD // 2], F16, name="sin_sb")

            # persistent SBUF across phases: transposed q/k, v, and Wo rows
            kv_pool = ctx.enter_context(tc.tile_pool(name="kv_pool", bufs=1))
            qt_sb = kv_pool.tile([P, NHL, S], F16, name="qt_sb")
            kt_sb = kv_pool.tile([P, NHL, S], F16, name="kt_sb")
            v_sb = kv_pool.tile([P, CT, GW], F16, name="v_sb")
            wo_sb = kv_pool.tile([P, NHL, D], F16, name="wo_sb")

            warm_rhs = const.tile([P, 512], F16, name="warm_rhs")
            nc.vector.memset(warm_rhs[:], 0.0)

            def norm_rope(rope, ps, ibg, t):
                """RMS-norm stats + RoPE on a projection PSUM tile; returns
                the rotated+normalized [P, GW] f16 tile."""
                qs = rope.tile([P, GW], F16, name=f"{t}s{ibg}", tag=f"{t}s")
                nc.scalar.copy(qs[:], ps[:])
                sq = rope.tile([P, GW], F16, name=f"{t}sq{ibg}", tag=f"{t}sq")
                nc.vector.tensor_mul(sq[:], qs[:], qs[:])
                rstd = rope.tile([P, NHL], F32, name=f"{t}rstd{ibg}", tag=f"{t}rstd")
                nc.vector.reduce_sum(
                    rstd[:],
                    sq[:].rearrange("p (h d) -> p h d", h=NHL),
                    axis=mybir.AxisListType.X,
                )
                nc.scalar.activation(
                    rstd[:], rstd[:],
                    mybir.ActivationFunctionType.Sqrt,
                    bias=eps_t[:], scale=1.0 / HD,
                )
                nc.vector.reciprocal(rstd[:], rstd[:])

                q3 = qs[:].rearrange("p (h d) -> p h d", h=NHL)
                qr = rope.tile([P, GW], F16, name=f"{t}r{ibg}", tag=f"{t}r")
                qr3 = qr[:].rearrange("p (h d) -> p h d", h=NHL)
                tmp = rope.tile([P, NHL, HD // 2], F16, name=f"{t}tmp{ibg}", tag=f"{t}tmp")
                cosB = cos_sb[:, ibg:ibg + 1, :].broadcast_to((P, NHL, HD // 2))
                sinB = sin_sb[:, ibg:ibg + 1, :].broadcast_to((P, NHL, HD // 2))
                h1 = q3[:, :, 0:HD // 2]
                h2 = q3[:, :, HD // 2:HD]
                # r1 = q1*cos + q2*sin ; r2 = q2*cos - q1*sin
                nc.vector.tensor_mul(qr3[:, :, 0:HD // 2], h1, cosB)
                nc.vector.tensor_mul(tmp[:], h2, sinB)
                nc.vector.tensor_add(qr3[:, :, 0:HD // 2], qr3[:, :, 0:HD // 2], tmp[:])
                nc.vector.tensor_mul(qr3[:, :, HD // 2:HD], h2, cosB)
                nc.vector.tensor_mul(tmp[:], h1, sinB)
                nc.vector.tensor_sub(
                    qr3[:, :, HD // 2:HD], qr3[:, :, HD // 2:HD], tmp[:]
                )
                for h in range(NHL):
                    nc.vector.tensor_scalar_mul(
                        qr[:, h * HD:(h + 1) * HD],
                        qr[:, h * HD:(h + 1) * HD],
                        rstd[:, h:h + 1],
                    )
                return qr

            # ---------------- Phase A: Q, K, V in one xt pass ----------------
            with ExitStack() as pha:
                wpool = pha.enter_context(tc.tile_pool(name="wpool", bufs=1))
                xt_pool = pha.enter_context(tc.tile_pool(name="xt_pool", bufs=2))
                proj_ps = pha.enter_context(tc.tile_pool(name="proj_ps", bufs=4, space="PSUM"))
                tp_ps = pha.enter_context(tc.tile_pool(name="tp_ps", bufs=2, space="PSUM"))
                rope = pha.enter_context(tc.tile_pool(name="rope", bufs=2))

                wq_sb = wpool.tile([P, CT, GW], F16, name="wq_sb")
                wk_sb = wpool.tile([P, CT, GW], F16, name="wk_sb")
                wv_sb = wpool.tile([P, CT, GW], F16, name="wv_sb")

                # PE warm-up during the DMA-dead startup window: keeps the
                # HAM clock gate warm so the first real matmuls run at 2.4GHz
                wps = proj_ps.tile([P, GW], F32, name="wps", tag="proj")
                for wi in range(12):
                    nc.tensor.matmul(wps[:], ident[:], warm_rhs[:],
                                     start=(wi == 0), stop=(wi == 11))

                # the two HW-DGE queues + scalar share HBM bandwidth roughly
                # per-queue; round-robin every tensor's slices across all
                # three, in global consumption order, so no single stream is
                # capped at 1/3 of the per-core bandwidth during startup
                rr = (nc.sync, nc.gpsimd, nc.scalar)

                def rr_w(wsb, w_in):
                    for ct in range(CT):
                        rr[ct % 3].dma_start(
                            out=wsb[:, ct, :], in_=w_in[ct * P:(ct + 1) * P, :])

                def xt_dma(ica, xt_ch, spread):
                    for ct in range(CT):
                        eng = rr[(ct + 1) % 3] if spread else nc.scalar
                        eng.dma_start(out=xt_ch[:, ct, :], in_=xt_in[ica, ct, :, :])

                def proj_pair(xt_ch, ibg0, wsb, t):
                    # both ib-halves interleaved per ct: each weight slice
                    # feeds two matmuls back-to-back, halving the startup
                    # DMA-bandwidth demand of the projection stream
                    pss = []
                    for ib in range(2):
                        pss.append(proj_ps.tile(
                            [P, GW], F32, name=f"ps{t}{ibg0 + ib}", tag="proj"))
                    for ct in range(CT):
                        for ib in range(2):
                            nc.tensor.matmul(
                                pss[ib][:], xt_ch[:, ct, ib * P:(ib + 1) * P],
                                wsb[:, ct, :],
                                start=(ct == 0), stop=(ct == CT - 1),
                            )
                    return pss

                def transposes(qr_or_kr, dst, ibg, t):
                    for h in range(NHL):
                        tp = tp_ps.tile([P, P], F16, name=f"tp{t}{ibg}_{h}", tag="tp")
                        nc.tensor.transpose(
                            tp[:], qr_or_kr[:, h * HD:(h + 1) * HD], ident[:])
                        nc.scalar.copy(dst[:, h, ibg * P:(ibg + 1) * P], tp[:])

                # ---- explicit schedule for xt chunks 0-1: v-projections are
                # deferred behind both chunks' q/k work (wv can arrive ~15us
                # later) and the PE transposes act as DMA-independent filler,
                # so the PE tracks the ramping HBM supply without long stalls
                xt_ch0 = xt_pool.tile([P, CT, 256], F16, name="xt_ch0", tag="xt")
                xt_ch1 = xt_pool.tile([P, CT, 256], F16, name="xt_ch1", tag="xt")
                for ct in range(CT):
                    rr[ct % 3].dma_start(
                        out=wq_sb[:, ct, :], in_=wq_in[ct * P:(ct + 1) * P, :])
                    rr[(ct + 1) % 3].dma_start(
                        out=xt_ch0[:, ct, :], in_=xt_in[0, ct, :, :])
                for ct in range(CT):
                    rr[ct % 3].dma_start(
                        out=wk_sb[:, ct, :], in_=wk_in[ct * P:(ct + 1) * P, :])
                    rr[(ct + 1) % 3].dma_start(
                        out=xt_ch1[:, ct, :], in_=xt_in[1, ct, :, :])
                nc.gpsimd.dma_start(out=cos_sb[:], in_=cos_in[:, :, :])
                nc.sync.dma_start(out=sin_sb[:], in_=sin_in[:, :, :])
                rr_w(wv_sb, wv_in)

                ps_q0 = proj_pair(xt_ch0, 0, wq_sb, "q")
                ps_k0 = proj_pair(xt_ch0, 0, wk_sb, "k")
                qr0 = [norm_rope(rope, ps_q0[ib], ib, "q") for ib in range(2)]
                kr0 = [norm_rope(rope, ps_k0[ib], ib, "k") for ib in range(2)]
                ps_q1 = proj_pair(xt_ch1, 2, wq_sb, "q")
                for ib in range(2):
                    transposes(qr0[ib], qt_sb, ib, "q")
                    transposes(kr0[ib], kt_sb, ib, "k")
                ps_k1 = proj_pair(xt_ch1, 2, wk_sb, "k")
                qr1 = [norm_rope(rope, ps_q1[ib], 2 + ib, "q") for ib in range(2)]
                ps_v0 = proj_pair(xt_ch0, 0, wv_sb, "v")
                kr1 = [norm_rope(rope, ps_k1[ib], 2 + ib, "k") for ib in range(2)]
                ps_v1 = proj_pair(xt_ch1, 2, wv_sb, "v")
                for ib in range(2):
                    nc.vector.tensor_copy(v_sb[:, ib, :], ps_v0[ib][:])
                    nc.vector.tensor_copy(v_sb[:, 2 + ib, :], ps_v1[ib][:])
                for ib in range(2):
                    transposes(qr1[ib], qt_sb, 2 + ib, "q")
                    transposes(kr1[ib], kt_sb, 2 + ib, "k")

                # ---- steady state: chunks 2-7 ----
                for ica in range(2, 8):
                    xt_ch = xt_pool.tile([P, CT, 256], F16, name=f"xt_ch{ica}", tag="xt")
                    xt_dma(ica, xt_ch, spread=(ica == 2))
                    if ica == 6:
                        # wo only needed by the first o_proj (~250us in)
                        for h in range(NHL):
                            nc.gpsimd.dma_start(
                                out=wo_sb[:, h, :],
                                in_=wo_in[h * P:(h + 1) * P, :],
                            )
                    ibg0 = 2 * ica
                    ps_q = proj_pair(xt_ch, ibg0, wq_sb, "q")
                    ps_k = proj_pair(xt_ch, ibg0, wk_sb, "k")
                    qr = [norm_rope(rope, ps_q[ib], ibg0 + ib, "q") for ib in range(2)]
                    ps_v = proj_pair(xt_ch, ibg0, wv_sb, "v")
                    kr = [norm_rope(rope, ps_k[ib], ibg0 + ib, "k") for ib in range(2)]
                    for ib in range(2):
                        nc.vector.tensor_copy(v_sb[:, ibg0 + ib, :], ps_v[ib][:])
                    for ib in range(2):
                        transposes(qr[ib], qt_sb, ibg0 + ib, "q")
                        transposes(kr[ib], kt_sb, ibg0 + ib, "k")

            # ------- Phase B: attention + per-chunk partial o_proj -------
            with ExitStack() as phb:
                et_pool = phb.enter_context(tc.tile_pool(name="et_pool", bufs=5))
                bsmall = phb.enter_context(tc.tile_pool(name="bsmall", bufs=2))
                # all 4 heads' normalized outputs stay live until the chunk's
                # o_proj consumes them
                ytsb_pool = phb.enter_context(tc.tile_pool(name="ytsb_pool", bufs=5))
                s_ps = phb.enter_context(tc.tile_pool(name="s_ps", bufs=4, space="PSUM"))
                yt_psp = phb.enter_context(tc.tile_pool(name="yt_psp", bufs=2, space="PSUM"))
                den_psp = phb.enter_context(tc.tile_pool(name="den_psp", bufs=2, space="PSUM"))

                for ic in range(ICH):
                    njb = 4 * ic + 4
                    pending_drain = None
                    ytn = {}
                    for h in range(NHL):
                        yt_ps = yt_psp.tile([P, 512], F32, name=f"yt{ic}_{h}", tag="yt")
                        den_ps = den_psp.tile([P, 512], F32, name=f"den{ic}_{h}", tag="den")
                        sps = {}

                        def emit_score(jb, h=h, ic=ic, sps=sps):
                            off = max(0, P * (jb - 4 * ic))
                            sp = s_ps.tile([P, 512], F32, name=f"s{ic}_{h}_{jb}", tag="s")
                            nc.tensor.matmul(
                                sp[:, off:512],
                                kt_sb[:, h, jb * P:(jb + 1) * P],
                                qt_sb[:, h, ic * 512 + off:(ic + 1) * 512],
                                start=True, stop=True,
                            )
                            sps[jb] = sp

                        def emit_finish(jb, h=h, ic=ic, njb=njb, sps=sps,
                                        yt_ps=yt_ps, den_ps=den_ps):
                            off = max(0, P * (jb - 4 * ic))
                            sp = sps.pop(jb)
                            et = et_pool.tile([P, 512], F16, name=f"et{ic}_{h}_{jb}", tag="et")
                            nc.scalar.activation(
                                et[:, off:512], sp[:, off:512],
                                mybir.ActivationFunctionType.Exp,
                                bias=neg1_t[:], scale=SCALE,
                            )
                            if jb >= 4 * ic:
                                # diagonal sub-block: zero the upper triangle
                                nc.vector.tensor_mul(
                                    et[:, off:off + P], et[:, off:off + P], tri_m[:]
                                )
                            nc.tensor.matmul(
                                yt_ps[:, off:512],
                                v_sb[:, jb, h * HD:(h + 1) * HD],
                                et[:, off:512],
                                start=(jb == 0), stop=(jb == njb - 1),
                            )
                            nc.tensor.matmul(
                                den_ps[:, off:512],
                                ones16[:],
                                et[:, off:512],
                                start=(jb == 0), stop=(jb == njb - 1),
                            )

                        # three scores of lookahead before the previous head's
                        # drain and before each finish
                        LA = min(3, njb)
                        for jb in range(LA):
                            emit_score(jb)
                        if pending_drain is not None:
                            pending_drain()
                            pending_drain = None
                        for jb in range(LA, njb):
                            emit_score(jb)
                            emit_finish(jb - LA)
                        for jb in range(njb - LA, njb):
                            emit_finish(jb)

                        def make_drain(h=h, ic=ic, yt_ps=yt_ps, den_ps=den_ps):
                            def drain():
                                # den is already replicated across partitions:
                                # one fast approx reciprocal + one multiply
                                rden = bsmall.tile([P, 512], F32, name=f"rd{ic}_{h}", tag="rden")
                                nc.vector.reciprocal_approx_fast(out=rden[:], in_=den_ps[:])
                                yt_sb = ytsb_pool.tile([P, 512], F16, name=f"yts{ic}_{h}", tag="yts")
                                nc.vector.tensor_mul(yt_sb[:], yt_ps[:], rden[:])
                                ytn[h] = yt_sb
                            return drain
                        if h == NHL - 1:
                            # the chunk's o_proj needs ytn[3] promptly
                            make_drain()()
                        else:
                            pending_drain = make_drain()

                    # ---- partial o_proj for this chunk (local heads only) ----
                    for oc in range(CT):
                        psp = yt_psp if oc % 2 == 0 else den_psp
                        y_ps = psp.tile([P, 512], F32, name=f"zp{ic}_{oc}",
                                        tag="yt" if oc % 2 == 0 else "den")
                        for j in range(NHL):
                            nc.tensor.matmul(
                                y_ps[:],
                                wo_sb[:, j, oc * P:(oc + 1) * P],
                                ytn[j][:],
                                start=(j == 0), stop=(j == NHL - 1),
                            )
                        y_sb = ytsb_pool.tile([P, 512], F16, name=f"zsb{ic}_{oc}", tag="zsb")
                        nc.vector.tensor_copy(y_sb[:], y_ps[:])
                        # sync is a HW-DGE queue and idle during phase B
                        # (gpsimd's queue is software-DGE: too slow here).
                        # the last chunk's stores are the exposed tail: split
                        # them across both HW queues (scalar is done by then)
                        eng = nc.scalar if (ic == ICH - 1 and oc % 2 == 1) else nc.sync
                        eng.dma_start(
                            out=z_out[oc, ic, :, :],
                            in_=y_sb[:],
                        )

    nc.compile()
    _program_cache["nc"] = nc
    return nc


def _rope_tables():
    inv_freq = 1.0 / (10000.0 ** (np.arange(0, HD, 2, dtype=np.float32) / HD))
    pos = np.arange(S, dtype=np.float32)
    freqs = np.outer(pos, inv_freq).astype(np.float32)
    cos = np.cos(freqs).astype(np.float16)
    sin = np.sin(freqs).astype(np.float16)
    # pretile [S, HD//2] -> [P, CT, HD//2] with [p, a, f] = t[a*128+p, f]
    cos_t = np.ascontiguousarray(cos.reshape(CT, P, HD // 2).transpose(1, 0, 2))
    sin_t = np.ascontiguousarray(sin.reshape(CT, P, HD // 2).transpose(1, 0, 2))
    return cos_t, sin_t


def make_in_maps(x, Wq, Wk, Wv, Wo):
    x = np.asarray(x, dtype=np.float32)
    cos_t, sin_t = _rope_tables()
    wqT = np.ascontiguousarray(np.asarray(Wq, dtype=np.float32).T.astype(np.float16))
    wkT = np.ascontiguousarray(np.asarray(Wk, dtype=np.float32).T.astype(np.float16))
    wvT = np.ascontiguousarray(np.asarray(Wv, dtype=np.float32).T.astype(np.float16))
    woT = np.ascontiguousarray(np.asarray(Wo, dtype=np.float32).T.astype(np.float16))
    # xT tiled to [ica, ct, p, f] so each (chunk, ct) DMA block is contiguous
    xts = [
        np.ascontiguousarray(
            x[b].T.astype(np.float16)
            .reshape(CT, P, 8, 256).transpose(2, 0, 1, 3)
        )
        for b in range(2)
    ]
    in_maps = []
    for c in range(NCORES):
        b, g = c // 4, c % 4
        sl = slice(g * GW, (g + 1) * GW)
        in_maps.append({
            "xt": xts[b],
            "wq": np.ascontiguousarray(wqT[:, sl]),
            "wk": np.ascontiguousarray(wkT[:, sl]),
            "wv": np.ascontiguousarray(wvT[:, sl]),
            "wo": np.ascontiguousarray(woT[sl, :]),
            "cos": cos_t,
            "sin": sin_t,
        })
    return in_maps


def assemble_output(results):
    y = np.empty((2, S, D), dtype=np.float32)
    for b in range(2):
        zt = np.zeros((CT, ICH, P, 512), dtype=np.float32)
        for g in range(4):
            zt += results[b * 4 + g]["z_out"].astype(np.float32)
        # [oc, ic, p, f] -> z[oc*128+p, ic*512+f] -> y = z.T
        z = zt.transpose(0, 2, 1, 3).reshape(D, S)
        y[b] = z.T
    return y


def kernel(x, Wq, Wk, Wv, Wo):
    nc = build_program()
    in_maps = make_in_maps(x, Wq, Wk, Wv, Wo)
    res = run_bass_kernel_spmd(nc, in_maps, core_ids=list(range(NCORES)))
    return assemble_output(res.results)


# revision 31
# speedup vs baseline: 1.0036x; 1.0036x over previous
"""Causal self-attention (QK-RMSNorm + RoPE) on 8 Trainium2 NeuronCores.

Problem: x[2,2048,2048], Wq/Wk/Wv/Wo [2048,2048], 16 heads, head_dim 128.

Sharding: core c handles batch b=c//4 and head group g=c%4 (4 heads,
model cols [512g:512g+512)).  No collectives: each core computes a
PARTIAL o_proj (contraction over its 4 local head dims only) and
returns zT_partial[2048, 2048]; the host sums the 4 partials per batch
and transposes.  This removes the AllGather tail (17-39us per 512KB
gather, serialized) entirely.

Phase A: Q/K/V projections in ONE pass over host-pre-transposed xT
(contraction dim on partitions), inputs streamed over three parallel
DMA rings.  wo/cos/sin do not compete for startup bandwidth: cos/sin
are host-pretiled to [128,16,64] (contiguous DMA, no SWDGE) and wo is
loaded at the last xt chunk (only needed ~250us in).  ~30 warm-up
matmuls on the identity run during the DMA-dead first ~8us so the HAM
clock gate is warm when the first real matmul issues.
q/k get RMS-norm + RoPE row-wise, are transposed per head on the PE,
and stay resident in SBUF (no DRAM round trip).

Phase B: attention with transposed scores (eT = exp(scale *
kT_blk.T @ qT_chunk)), so the AV matmul (lhsT=v, rhs=eT) directly
yields the transposed output yT[d, i].  Each key block only streams
its causally valid query window (128-granular); a single 128x128
triangular tile masks the diagonal sub-blocks.  Scores are emitted 3
blocks ahead of their exp/AV/den consumers.  The softmax denominator
matmul uses a full ones[128,128] stationary operand so the column
sums come out replicated across all partitions; the drain is then
reciprocal_approx_fast + one multiply on the vector engine.

After each i-chunk's 4 heads are normalized (ytn in SBUF), the
partial o_proj for that chunk runs immediately (64 matmuls:
16 o-tiles x 4 local head contractions) and its z tiles stream out to
DRAM -- no inter-core dependency, so the PE never waits on a
collective.
"""

import math
from contextlib import ExitStack

import numpy as np

import concourse.bass as bass
import concourse.bacc as bacc
import concourse.tile as tile
from concourse import mybir
from concourse.bass_utils import run_bass_kernel_spmd
from concourse.masks import make_identity

P = 128
D = 2048
S = 2048
HD = 128              # head dim
NHL = 4               # heads per core
GW = NHL * HD         # 512, per-core width of head group
CT = D // P           # 16 contraction tiles
ICH = 4               # i-chunks of 512 positions
NCORES = 8
F32 = mybir.dt.float32
F16 = mybir.dt.float16
SCALE = 1.0 / math.sqrt(HD)
EPS = 1.1920928955078125e-07

_program_cache = {}


def build_program():
    if "nc" in _program_cache:
        return _program_cache["nc"]

    nc = bacc.Bacc("TRN2", target_bir_lowering=False, debug=False, num_devices=NCORES)

    # host-pretiled xT: [ica, ct, p, f] = xT[ct*128+p, ica*256+f] so every
    # (chunk, ct) DMA slice is a contiguous 64KB block (strided 512B-segment
    # reads starved the PE during startup)
    xt_in = nc.dram_tensor("xt", [8, CT, P, 256], F16, kind="ExternalInput")
    wq_in = nc.dram_tensor("wq", [D, GW], F16, kind="ExternalInput")
    wk_in = nc.dram_tensor("wk", [D, GW], F16, kind="ExternalInput")
    wv_in = nc.dram_tensor("wv", [D, GW], F16, kind="ExternalInput")
    # Wo.T row slice [512, 2048]: local head dims x full model width
    wo_in = nc.dram_tensor("wo", [GW, D], F16, kind="ExternalInput")
    # host-pretiled rope tables: [p, a, f] = table[a*128+p, f]
    cos_in = nc.dram_tensor("cos", [P, CT, HD // 2], F16, kind="ExternalInput")
    sin_in = nc.dram_tensor("sin", [P, CT, HD // 2], F16, kind="ExternalInput")
    # partial o_proj output, tiled [oc, ic, p, f] = z[oc*128+p, ic*512+f]
    # (contiguous 128KB writes); host sums over 4 cores and untiles
    z_out = nc.dram_tensor("z_out", [CT, ICH, P, 512], F16, kind="ExternalOutput")

    with tile.TileContext(nc) as tc:
        with ExitStack() as ctx:
            const = ctx.enter_context(tc.tile_pool(name="const", bufs=1))

            ident = const.tile([P, P], F16, name="ident")
            make_identity(nc, ident)
            eps_t = const.tile([P, 1], F32, name="eps_t")
            nc.vector.memset(eps_t[:], EPS)
            neg1_t = const.tile([P, 1], F32, name="neg1_t")
            nc.vector.memset(neg1_t[:], -1.0)
            ones_f = const.tile([P, P], F32, name="ones_f")
            nc.vector.memset(ones_f[:], 1.0)
            # full-width ones: den matmul replicates the column sums across
            # all 128 output partitions (same stream cost as 2 partitions)
            ones16 = const.tile([P, P], F16, name="ones16")
            nc.scalar.copy(ones16[:], ones_f[:])
            # tri_m[j, i] = 1 where i >= j (valid), else 0 — diagonal block mask
            tri_m = const.tile([P, P], F16, name="tri_m")
            nc.gpsimd.memset(tri_m[:], 1.0)
            nc.gpsimd.affine_select(
                out=tri_m[:], in_=tri_m[:],
                compare_op=mybir.AluOpType.is_ge,
                fill=0.0,
                base=0,
                pattern=[[1, P]],
                channel_multiplier=-1,
            )

            cos_sb = const.tile([P, CT, HD // 2], F16, name="cos_sb")
            sin_sb = const.tile([P, CT, HD // 2], F16, name="sin_sb")

            # persistent SBUF across phases: transposed q/k, v, and Wo rows
            kv_pool = ctx.enter_context(tc.tile_pool(name="kv_pool", bufs=1))
            qt_sb = kv_pool.tile([P, NHL, S], F16, name="qt_sb")
            kt_sb = kv_pool.tile([P, NHL, S], F16, name="kt_sb")
            v_sb = kv_pool.tile([P, CT, GW], F16, name="v_sb")
            wo_sb = kv_pool.tile([P, NHL, D], F16, name="wo_sb")

            warm_rhs = const.tile([P, 512], F16, name="warm_rhs")
            nc.vector.memset(warm_rhs[:], 0.0)

            def norm_rope(rope, ps, ibg, t):
                """RMS-norm stats + RoPE on a projection PSUM tile; returns
                the rotated+normalized [P, GW] f16 tile."""
                qs = rope.tile([P, GW], F16, name=f"{t}s{ibg}", tag=f"{t}s")
                nc.scalar.copy(qs[:], ps[:])
                sq = rope.tile([P, GW], F16, name=f"{t}sq{ibg}", tag=f"{t}sq")
                nc.vector.tensor_mul(sq[:], qs[:], qs[:])
                rstd = rope.tile([P, NHL], F32, name=f"{t}rstd{ibg}", tag=f"{t}rstd")
                nc.vector.reduce_sum(
                    rstd[:],
                    sq[:].rearrange("p (h d) -> p h d", h=NHL),
                    axis=mybir.AxisListType.X,
                )
                nc.scalar.activation(
                    rstd[:], rstd[:],
                    mybir.ActivationFunctionType.Sqrt,
                    bias=eps_t[:], scale=1.0 / HD,
                )
                nc.vector.reciprocal(rstd[:], rstd[:])

                q3 = qs[:].rearrange("p (h d) -> p h d", h=NHL)
                qr = rope.tile([P, GW], F16, name=f"{t}r{ibg}", tag=f"{t}r")
                qr3 = qr[:].rearrange("p (h d) -> p h d", h=NHL)
                tmp = rope.tile([P, NHL, HD // 2], F16, name=f"{t}tmp{ibg}", tag=f"{t}tmp")
                cosB = cos_sb[:, ibg:ibg + 1, :].broadcast_to((P, NHL, HD // 2))
                sinB = sin_sb[:, ibg:ibg + 1, :].broadcast_to((P, NHL, HD // 2))
                h1 = q3[:, :, 0:HD // 2]
                h2 = q3[:, :, HD // 2:HD]
                # r1 = q1*cos + q2*sin ; r2 = q2*cos - q1*sin
                nc.vector.tensor_mul(qr3[:, :, 0:HD // 2], h1, cosB)
                nc.vector.tensor_mul(tmp[:], h2, sinB)
                nc.vector.tensor_add(qr3[:, :, 0:HD // 2], qr3[:, :, 0:HD // 2], tmp[:])
                nc.vector.tensor_mul(qr3[:, :, HD // 2:HD], h2, cosB)
                nc.vector.tensor_mul(tmp[:], h1, sinB)
                nc.vector.tensor_sub(
                    qr3[:, :, HD // 2:HD], qr3[:, :, HD // 2:HD], tmp[:]
                )
                for h in range(NHL):
                    nc.vector.tensor_scalar_mul(
                        qr[:, h * HD:(h + 1) * HD],
                        qr[:, h * HD:(h + 1) * HD],
                        rstd[:, h:h + 1],
                    )
                return qr

            # ---------------- Phase A: Q, K, V in one xt pass ----------------
            with ExitStack() as pha:
                wpool = pha.enter_context(tc.tile_pool(name="wpool", bufs=1))
                xt_pool = pha.enter_context(tc.tile_pool(name="xt_pool", bufs=2))
                proj_ps = pha.enter_context(tc.tile_pool(name="proj_ps", bufs=4, space="PSUM"))
                tp_ps = pha.enter_context(tc.tile_pool(name="tp_ps", bufs=2, space="PSUM"))
                rope = pha.enter_context(tc.tile_pool(name="rope", bufs=2))

                wq_sb = wpool.tile([P, CT, GW], F16, name="wq_sb")
                wk_sb = wpool.tile([P, CT, GW], F16, name="wk_sb")
                wv_sb = wpool.tile([P, CT, GW], F16, name="wv_sb")

                # PE warm-up during the DMA-dead startup window: keeps the
                # HAM clock gate warm so the first real matmuls run at 2.4GHz
                wps = proj_ps.tile([P, GW], F32, name="wps", tag="proj")
                for wi in range(12):
                    nc.tensor.matmul(wps[:], ident[:], warm_rhs[:],
                                     start=(wi == 0), stop=(wi == 11))

                # the two HW-DGE queues + scalar share HBM bandwidth roughly
                # per-queue; round-robin every tensor's slices across all
                # three, in global consumption order, so no single stream is
                # capped at 1/3 of the per-core bandwidth during startup
                rr = (nc.sync, nc.gpsimd, nc.scalar)

                def xt_dmas(ica, xt_ch):
                    if ica == 0:
                        for ct in range(CT):
                            rr[ct % 3].dma_start(
                                out=wq_sb[:, ct, :], in_=wq_in[ct * P:(ct + 1) * P, :])
                            rr[(ct + 1) % 3].dma_start(
                                out=xt_ch[:, ct, :], in_=xt_in[ica, ct, :, :])
                        for ct in range(CT):
                            rr[ct % 3].dma_start(
                                out=wk_sb[:, ct, :], in_=wk_in[ct * P:(ct + 1) * P, :])
                        nc.gpsimd.dma_start(out=cos_sb[:], in_=cos_in[:, :, :])
                        nc.sync.dma_start(out=sin_sb[:], in_=sin_in[:, :, :])
                        for ct in range(CT):
                            rr[ct % 3].dma_start(
                                out=wv_sb[:, ct, :], in_=wv_in[ct * P:(ct + 1) * P, :])
                        return
                    for ct in range(CT):
                        eng = rr[(ct + 2) % 3] if ica <= 2 else nc.scalar
                        eng.dma_start(out=xt_ch[:, ct, :], in_=xt_in[ica, ct, :, :])
                    if ica == 6:
                        # wo only needed by the first o_proj (~250us in)
                        for h in range(NHL):
                            nc.gpsimd.dma_start(
                                out=wo_sb[:, h, :],
                                in_=wo_in[h * P:(h + 1) * P, :],
                            )

                for ica in range(8):
                    xt_ch = xt_pool.tile([P, CT, 256], F16, name=f"xt_ch{ica}", tag="xt")
                    xt_dmas(ica, xt_ch)
                    ibg0 = 2 * ica

                    # both ib-halves interleaved per ct: each weight slice
                    # feeds two matmuls back-to-back, halving the startup
                    # DMA-bandwidth demand of the projection stream
                    def proj_pair(wsb, t):
                        pss = []
                        for ib in range(2):
                            pss.append(proj_ps.tile(
                                [P, GW], F32, name=f"ps{t}{ibg0 + ib}", tag="proj"))
                        for ct in range(CT):
                            for ib in range(2):
                                nc.tensor.matmul(
                                    pss[ib][:], xt_ch[:, ct, ib * P:(ib + 1) * P],
                                    wsb[:, ct, :],
                                    start=(ct == 0), stop=(ct == CT - 1),
                                )
                        return pss

                    ps_q = proj_pair(wq_sb, "q")
                    ps_k = proj_pair(wk_sb, "k")
                    qr = [norm_rope(rope, ps_q[ib], ibg0 + ib, "q") for ib in range(2)]
                    ps_v = proj_pair(wv_sb, "v")
                    kr = [norm_rope(rope, ps_k[ib], ibg0 + ib, "k") for ib in range(2)]
                    for ib in range(2):
                        nc.vector.tensor_copy(v_sb[:, ibg0 + ib, :], ps_v[ib][:])
                    for ib in range(2):
                        ibg = ibg0 + ib
                        for h in range(NHL):
                            tp = tp_ps.tile([P, P], F16, name=f"tpq{ibg}_{h}", tag="tp")
                            nc.tensor.transpose(tp[:], qr[ib][:, h * HD:(h + 1) * HD], ident[:])
                            nc.scalar.copy(qt_sb[:, h, ibg * P:(ibg + 1) * P], tp[:])
                        for h in range(NHL):
                            tp = tp_ps.tile([P, P], F16, name=f"tpk{ibg}_{h}", tag="tp")
                            nc.tensor.transpose(tp[:], kr[ib][:, h * HD:(h + 1) * HD], ident[:])
                            nc.scalar.copy(kt_sb[:, h, ibg * P:(ibg + 1) * P], tp[:])

            # ------- Phase B: attention + per-chunk partial o_proj -------
            with ExitStack() as phb:
                et_pool = phb.enter_context(tc.tile_pool(name="et_pool", bufs=5))
                bsmall = phb.enter_context(tc.tile_pool(name="bsmall", bufs=2))
                # all 4 heads' normalized outputs stay live until the chunk's
                # o_proj consumes them
                ytsb_pool = phb.enter_context(tc.tile_pool(name="ytsb_pool", bufs=5))
                s_ps = phb.enter_context(tc.tile_pool(name="s_ps", bufs=4, space="PSUM"))
                yt_psp = phb.enter_context(tc.tile_pool(name="yt_psp", bufs=2, space="PSUM"))
                den_psp = phb.enter_context(tc.tile_pool(name="den_psp", bufs=2, space="PSUM"))

                for ic in range(ICH):
                    njb = 4 * ic + 4
                    pending_drain = None
                    ytn = {}
                    for h in range(NHL):
                        yt_ps = yt_psp.tile([P, 512], F32, name=f"yt{ic}_{h}", tag="yt")
                        den_ps = den_psp.tile([P, 512], F32, name=f"den{ic}_{h}", tag="den")
                        sps = {}

                        def emit_score(jb, h=h, ic=ic, sps=sps):
                            off = max(0, P * (jb - 4 * ic))
                            sp = s_ps.tile([P, 512], F32, name=f"s{ic}_{h}_{jb}", tag="s")
                            nc.tensor.matmul(
                                sp[:, off:512],
                                kt_sb[:, h, jb * P:(jb + 1) * P],
                                qt_sb[:, h, ic * 512 + off:(ic + 1) * 512],
                                start=True, stop=True,
                            )
                            sps[jb] = sp

                        def emit_finish(jb, h=h, ic=ic, njb=njb, sps=sps,
                                        yt_ps=yt_ps, den_ps=den_ps):
                            off = max(0, P * (jb - 4 * ic))
                            sp = sps.pop(jb)
                            et = et_pool.tile([P, 512], F16, name=f"et{ic}_{h}_{jb}", tag="et")
                            nc.scalar.activation(
                                et[:, off:512], sp[:, off:512],
                                mybir.ActivationFunctionType.Exp,
                                bias=neg1_t[:], scale=SCALE,
                            )
                            if jb >= 4 * ic:
                                # diagonal sub-block: zero the upper triangle
                                nc.vector.tensor_mul(
                                    et[:, off:off + P], et[:, off:off + P], tri_m[:]
                                )
                            nc.tensor.matmul(
                                yt_ps[:, off:512],
                                v_sb[:, jb, h * HD:(h + 1) * HD],
                                et[:, off:512],
                                start=(jb == 0), stop=(jb == njb - 1),
                            )
                            nc.tensor.matmul(
                                den_ps[:, off:512],
                                ones16[:],
                                et[:, off:512],
                                start=(jb == 0), stop=(jb == njb - 1),
                            )

                        # three scores of lookahead before the previous head's
                        # drain and before each finish
                        LA = min(3, njb)
                        for jb in range(LA):
                            emit_score(jb)
                        if pending_drain is not None:
                            pending_drain()
                            pending_drain = None
                        for jb in range(LA, njb):
                            emit_score(jb)
                            emit_finish(jb - LA)
                        for jb in range(njb - LA, njb):
                            emit_finish(jb)

                        def make_drain(h=h, ic=ic, yt_ps=yt_ps, den_ps=den_ps):
                            def drain():
                                # den is already replicated across partitions:
                                # one fast approx reciprocal + one multiply
                                rden = bsmall.tile([P, 512], F32, name=f"rd{ic}_{h}", tag="rden")
                                nc.vector.reciprocal_approx_fast(out=rden[:], in_=den_ps[:])
                                yt_sb = ytsb_pool.tile([P, 512], F16, name=f"yts{ic}_{h}", tag="yts")
                                nc.vector.tensor_mul(yt_sb[:], yt_ps[:], rden[:])
                                ytn[h] = yt_sb
                            return drain
                        if h == NHL - 1:
                            # the chunk's o_proj needs ytn[3] promptly
                            make_drain()()
                        else:
                            pending_drain = make_drain()

                    # ---- partial o_proj for this chunk (local heads only) ----
                    for oc in range(CT):
                        psp = yt_psp if oc % 2 == 0 else den_psp
                        y_ps = psp.tile([P, 512], F32, name=f"zp{ic}_{oc}",
                                        tag="yt" if oc % 2 == 0 else "den")
                        for j in range(NHL):
                            nc.tensor.matmul(
                                y_ps[:],
                                wo_sb[:, j, oc * P:(oc + 1) * P],
                                ytn[j][:],
                                start=(j == 0), stop=(j == NHL - 1),
                            )
                        y_sb = ytsb_pool.tile([P, 512], F16, name=f"zsb{ic}_{oc}", tag="zsb")
                        nc.vector.tensor_copy(y_sb[:], y_ps[:])
                        # sync is a HW-DGE queue and idle during phase B
                        # (gpsimd's queue is software-DGE: too slow here)
                        nc.sync.dma_start(
                            out=z_out[oc, ic, :, :],
                            in_=y_sb[:],
                        )

    nc.compile()
    _program_cache["nc"] = nc
    return nc


def _rope_tables():
    inv_freq = 1.0 / (10000.0 ** (np.arange(0, HD, 2, dtype=np.float32) / HD))
    pos = np.arange(S, dtype=np.float32)
    freqs = np.outer(pos, inv_freq).astype(np.float32)
    cos = np.cos(freqs).astype(np.float16)
    sin = np.sin(freqs).astype(np.float16)
    # pretile [S, HD//2] -> [P, CT, HD//2] with [p, a, f] = t[a*128+p, f]
    cos_t = np.ascontiguousarray(cos.reshape(CT, P, HD // 2).transpose(1, 0, 2))
    sin_t = np.ascontiguousarray(sin.reshape(CT, P, HD // 2).transpose(1, 0, 2))
    return cos_t, sin_t


def make_in_maps(x, Wq, Wk, Wv, Wo):
    x = np.asarray(x, dtype=np.float32)
    cos_t, sin_t = _rope_tables()
    wqT = np.ascontiguousarray(np.asarray(Wq, dtype=np.float32).T.astype(np.float16))
    wkT = np.ascontiguousarray(np.asarray(Wk, dtype=np.float32).T.astype(np.float16))
    wvT = np.ascontiguousarray(np.asarray(Wv, dtype=np.float32).T.astype(np.float16))
    woT = np.ascontiguousarray(np.asarray(Wo, dtype=np.float32).T.astype(np.float16))
    # xT tiled to [ica, ct, p, f] so each (chunk, ct) DMA block is contiguous
    xts = [
        np.ascontiguousarray(
            x[b].T.astype(np.float16)
            .reshape(CT, P, 8, 256).transpose(2, 0, 1, 3)
        )
        for b in range(2)
    ]
    in_maps = []
    for c in range(NCORES):
        b, g = c // 4, c % 4
        sl = slice(g * GW, (g + 1) * GW)
        in_maps.append({
            "xt": xts[b],
            "wq": np.ascontiguousarray(wqT[:, sl]),
            "wk": np.ascontiguousarray(wkT[:, sl]),
            "wv": np.ascontiguousarray(wvT[:, sl]),
            "wo": np.ascontiguousarray(woT[sl, :]),
            "cos": cos_t,
            "sin": sin_t,
        })
    return in_maps


def assemble_output(results):
    y = np.empty((2, S, D), dtype=np.float32)
    for b in range(2):
        zt = np.zeros((CT, ICH, P, 512), dtype=np.float32)
        for g in range(4):
            zt += results[b * 4 + g]["z_out"].astype(np.float32)
        # [oc, ic, p, f] -> z[oc*128+p, ic*512+f] -> y = z.T
        z = zt.transpose(0, 2, 1, 3).reshape(D, S)
        y[b] = z.T
    return y


def kernel(x, Wq, Wk, Wv, Wo):
    nc = build_program()
    in_maps = make_in_maps(x, Wq, Wk, Wv, Wo)
    res = run_bass_kernel_spmd(nc, in_maps, core_ids=list(range(NCORES)))
    return assemble_output(res.results)


# revision 35
# speedup vs baseline: 1.0037x; 1.0002x over previous
"""Causal self-attention (QK-RMSNorm + RoPE) on 8 Trainium2 NeuronCores.

Problem: x[2,2048,2048], Wq/Wk/Wv/Wo [2048,2048], 16 heads, head_dim 128.

Sharding: core c handles batch b=c//4 and head group g=c%4 (4 heads,
model cols [512g:512g+512)).  No collectives: each core computes a
PARTIAL o_proj (contraction over its 4 local head dims only) and
returns zT_partial[2048, 2048]; the host sums the 4 partials per batch
and transposes.  This removes the AllGather tail (17-39us per 512KB
gather, serialized) entirely.

Phase A: Q/K/V projections in ONE pass over host-pre-transposed xT
(contraction dim on partitions), inputs streamed over three parallel
DMA rings.  wo/cos/sin do not compete for startup bandwidth: cos/sin
are host-pretiled to [128,16,64] (contiguous DMA, no SWDGE) and wo is
loaded at the last xt chunk (only needed ~250us in).  ~30 warm-up
matmuls on the identity run during the DMA-dead first ~8us so the HAM
clock gate is warm when the first real matmul issues.
q/k get RMS-norm + RoPE row-wise, are transposed per head on the PE,
and stay resident in SBUF (no DRAM round trip).

Phase B: attention with transposed scores (eT = exp(scale *
kT_blk.T @ qT_chunk)), so the AV matmul (lhsT=v, rhs=eT) directly
yields the transposed output yT[d, i].  Each key block only streams
its causally valid query window (128-granular); a single 128x128
triangular tile masks the diagonal sub-blocks.  Scores are emitted 3
blocks ahead of their exp/AV/den consumers.  The softmax denominator
matmul uses a full ones[128,128] stationary operand so the column
sums come out replicated across all partitions; the drain is then
reciprocal_approx_fast + one multiply on the vector engine.

After each i-chunk's 4 heads are normalized (ytn in SBUF), the
partial o_proj for that chunk runs immediately (64 matmuls:
16 o-tiles x 4 local head contractions) and its z tiles stream out to
DRAM -- no inter-core dependency, so the PE never waits on a
collective.
"""

import math
from contextlib import ExitStack

import numpy as np

import concourse.bass as bass
import concourse.bacc as bacc
import concourse.tile as tile
from concourse import mybir
from concourse.bass_utils import run_bass_kernel_spmd
from concourse.masks import make_identity

P = 128
D = 2048
S = 2048
HD = 128              # head dim
NHL = 4               # heads per core
GW = NHL * HD         # 512, per-core width of head group
CT = D // P           # 16 contraction tiles
ICH = 4               # i-chunks of 512 positions
NCORES = 8
F32 = mybir.dt.float32
F16 = mybir.dt.float16
SCALE = 1.0 / math.sqrt(HD)
EPS = 1.1920928955078125e-07

_program_cache = {}


def build_program():
    if "nc" in _program_cache:
        return _program_cache["nc"]

    nc = bacc.Bacc("TRN2", target_bir_lowering=False, debug=False, num_devices=NCORES)

    # host-pretiled xT: [ica, ct, p, f] = xT[ct*128+p, ica*256+f] so every
    # (chunk, ct) DMA slice is a contiguous 64KB block (strided 512B-segment
    # reads starved the PE during startup)
    xt_in = nc.dram_tensor("xt", [8, CT, P, 256], F16, kind="ExternalInput")
    wq_in = nc.dram_tensor("wq", [D, GW], F16, kind="ExternalInput")
    wk_in = nc.dram_tensor("wk", [D, GW], F16, kind="ExternalInput")
    wv_in = nc.dram_tensor("wv", [D, GW], F16, kind="ExternalInput")
    # Wo.T row slice [512, 2048]: local head dims x full model width
    wo_in = nc.dram_tensor("wo", [GW, D], F16, kind="ExternalInput")
    # host-pretiled rope tables: [p, a, f] = table[a*128+p, f]
    cos_in = nc.dram_tensor("cos", [P, CT, HD // 2], F16, kind="ExternalInput")
    sin_in = nc.dram_tensor("sin", [P, CT, HD // 2], F16, kind="ExternalInput")
    # partial o_proj output, tiled [oc, ic, p, f] = z[oc*128+p, ic*512+f]
    # (contiguous 128KB writes); host sums over 4 cores and untiles
    z_out = nc.dram_tensor("z_out", [CT, ICH, P, 512], F16, kind="ExternalOutput")

    with tile.TileContext(nc) as tc:
        with ExitStack() as ctx:
            const = ctx.enter_context(tc.tile_pool(name="const", bufs=1))

            ident = const.tile([P, P], F16, name="ident")
            make_identity(nc, ident)
            eps_t = const.tile([P, 1], F32, name="eps_t")
            nc.vector.memset(eps_t[:], EPS)
            neg1_t = const.tile([P, 1], F32, name="neg1_t")
            nc.vector.memset(neg1_t[:], -1.0)
            ones_f = const.tile([P, P], F32, name="ones_f")
            nc.vector.memset(ones_f[:], 1.0)
            # full-width ones: den matmul replicates the column sums across
            # all 128 output partitions (same stream cost as 2 partitions)
            ones16 = const.tile([P, P], F16, name="ones16")
            nc.scalar.copy(ones16[:], ones_f[:])
            # tri_m[j, i] = 1 where i >= j (valid), else 0 — diagonal block mask
            tri_m = const.tile([P, P], F16, name="tri_m")
            nc.gpsimd.memset(tri_m[:], 1.0)
            nc.gpsimd.affine_select(
                out=tri_m[:], in_=tri_m[:],
                compare_op=mybir.AluOpType.is_ge,
                fill=0.0,
                base=0,
                pattern=[[1, P]],
                channel_multiplier=-1,
            )

            cos_sb = const.tile([P, CT, HD // 2], F16, name="cos_sb")
            sin_sb = const.tile([P, CT, HD // 2], F16, name="sin_sb")

            # persistent SBUF across phases: transposed q/k, v, and Wo rows
            kv_pool = ctx.enter_context(tc.tile_pool(name="kv_pool", bufs=1))
            qt_sb = kv_pool.tile([P, NHL, S], F16, name="qt_sb")
            kt_sb = kv_pool.tile([P, NHL, S], F16, name="kt_sb")
            v_sb = kv_pool.tile([P, CT, GW], F16, name="v_sb")
            wo_sb = kv_pool.tile([P, NHL, D], F16, name="wo_sb")

            warm_rhs = const.tile([P, 512], F16, name="warm_rhs")
            nc.vector.memset(warm_rhs[:], 0.0)

            # phase B's SBUF pools live alongside phase A's (SBUF has room),
            # so the A->B transition only waits on the PSUM pool swap
            et_pool = ctx.enter_context(tc.tile_pool(name="et_pool", bufs=5))
            bsmall = ctx.enter_context(tc.tile_pool(name="bsmall", bufs=2))
            # all 4 heads' normalized outputs stay live until the chunk's
            # o_proj consumes them
            ytsb_pool = ctx.enter_context(tc.tile_pool(name="ytsb_pool", bufs=5))

            def norm_rope(rope, ps, ibg, t):
                """RMS-norm stats + RoPE on a projection PSUM tile; returns
                the rotated+normalized [P, GW] f16 tile."""
                qs = rope.tile([P, GW], F16, name=f"{t}s{ibg}", tag=f"{t}s")
                nc.scalar.copy(qs[:], ps[:])
                sq = rope.tile([P, GW], F16, name=f"{t}sq{ibg}", tag=f"{t}sq")
                nc.vector.tensor_mul(sq[:], qs[:], qs[:])
                rstd = rope.tile([P, NHL], F32, name=f"{t}rstd{ibg}", tag=f"{t}rstd")
                nc.vector.reduce_sum(
                    rstd[:],
                    sq[:].rearrange("p (h d) -> p h d", h=NHL),
                    axis=mybir.AxisListType.X,
                )
                nc.scalar.activation(
                    rstd[:], rstd[:],
                    mybir.ActivationFunctionType.Sqrt,
                    bias=eps_t[:], scale=1.0 / HD,
                )
                nc.vector.reciprocal(rstd[:], rstd[:])

                q3 = qs[:].rearrange("p (h d) -> p h d", h=NHL)
                qr = rope.tile([P, GW], F16, name=f"{t}r{ibg}", tag=f"{t}r")
                qr3 = qr[:].rearrange("p (h d) -> p h d", h=NHL)
                tmp = rope.tile([P, NHL, HD // 2], F16, name=f"{t}tmp{ibg}", tag=f"{t}tmp")
                cosB = cos_sb[:, ibg:ibg + 1, :].broadcast_to((P, NHL, HD // 2))
                sinB = sin_sb[:, ibg:ibg + 1, :].broadcast_to((P, NHL, HD // 2))
                h1 = q3[:, :, 0:HD // 2]
                h2 = q3[:, :, HD // 2:HD]
                # r1 = q1*cos + q2*sin ; r2 = q2*cos - q1*sin
                nc.vector.tensor_mul(qr3[:, :, 0:HD // 2], h1, cosB)
                nc.vector.tensor_mul(tmp[:], h2, sinB)
                nc.vector.tensor_add(qr3[:, :, 0:HD // 2], qr3[:, :, 0:HD // 2], tmp[:])
                nc.vector.tensor_mul(qr3[:, :, HD // 2:HD], h2, cosB)
                nc.vector.tensor_mul(tmp[:], h1, sinB)
                nc.vector.tensor_sub(
                    qr3[:, :, HD // 2:HD], qr3[:, :, HD // 2:HD], tmp[:]
                )
                for h in range(NHL):
                    nc.vector.tensor_scalar_mul(
                        qr[:, h * HD:(h + 1) * HD],
                        qr[:, h * HD:(h + 1) * HD],
                        rstd[:, h:h + 1],
                    )
                return qr

            # ---------------- Phase A: Q, K, V in one xt pass ----------------
            with ExitStack() as pha:
                wpool = pha.enter_context(tc.tile_pool(name="wpool", bufs=1))
                xt_pool = pha.enter_context(tc.tile_pool(name="xt_pool", bufs=2))
                proj_ps = pha.enter_context(tc.tile_pool(name="proj_ps", bufs=4, space="PSUM"))
                tp_ps = pha.enter_context(tc.tile_pool(name="tp_ps", bufs=2, space="PSUM"))
                rope = pha.enter_context(tc.tile_pool(name="rope", bufs=2))

                wq_sb = wpool.tile([P, CT, GW], F16, name="wq_sb")
                wk_sb = wpool.tile([P, CT, GW], F16, name="wk_sb")
                wv_sb = wpool.tile([P, CT, GW], F16, name="wv_sb")

                # PE warm-up during the DMA-dead startup window: keeps the
                # HAM clock gate warm so the first real matmuls run at 2.4GHz
                wps = proj_ps.tile([P, GW], F32, name="wps", tag="proj")
                for wi in range(12):
                    nc.tensor.matmul(wps[:], ident[:], warm_rhs[:],
                                     start=(wi == 0), stop=(wi == 11))

                # the two HW-DGE queues + scalar share HBM bandwidth roughly
                # per-queue; round-robin every tensor's slices across all
                # three, in global consumption order, so no single stream is
                # capped at 1/3 of the per-core bandwidth during startup
                rr = (nc.sync, nc.gpsimd, nc.scalar)

                def xt_dmas(ica, xt_ch):
                    if ica == 0:
                        for ct in range(CT):
                            rr[ct % 3].dma_start(
                                out=wq_sb[:, ct, :], in_=wq_in[ct * P:(ct + 1) * P, :])
                            rr[(ct + 1) % 3].dma_start(
                                out=xt_ch[:, ct, :], in_=xt_in[ica, ct, :, :])
                        for ct in range(CT):
                            rr[ct % 3].dma_start(
                                out=wk_sb[:, ct, :], in_=wk_in[ct * P:(ct + 1) * P, :])
                        nc.gpsimd.dma_start(out=cos_sb[:], in_=cos_in[:, :, :])
                        nc.sync.dma_start(out=sin_sb[:], in_=sin_in[:, :, :])
                        for ct in range(CT):
                            rr[ct % 3].dma_start(
                                out=wv_sb[:, ct, :], in_=wv_in[ct * P:(ct + 1) * P, :])
                        return
                    for ct in range(CT):
                        eng = rr[(ct + 2) % 3] if ica <= 2 else nc.scalar
                        eng.dma_start(out=xt_ch[:, ct, :], in_=xt_in[ica, ct, :, :])
                    if ica == 6:
                        # wo only needed by the first o_proj (~250us in)
                        for h in range(NHL):
                            nc.gpsimd.dma_start(
                                out=wo_sb[:, h, :],
                                in_=wo_in[h * P:(h + 1) * P, :],
                            )

                for ica in range(8):
                    xt_ch = xt_pool.tile([P, CT, 256], F16, name=f"xt_ch{ica}", tag="xt")
                    xt_dmas(ica, xt_ch)
                    ibg0 = 2 * ica

                    # both ib-halves interleaved per ct: each weight slice
                    # feeds two matmuls back-to-back, halving the startup
                    # DMA-bandwidth demand of the projection stream
                    def proj_pair(wsb, t):
                        pss = []
                        for ib in range(2):
                            pss.append(proj_ps.tile(
                                [P, GW], F32, name=f"ps{t}{ibg0 + ib}", tag="proj"))
                        for ct in range(CT):
                            for ib in range(2):
                                nc.tensor.matmul(
                                    pss[ib][:], xt_ch[:, ct, ib * P:(ib + 1) * P],
                                    wsb[:, ct, :],
                                    start=(ct == 0), stop=(ct == CT - 1),
                                )
                        return pss

                    ps_q = proj_pair(wq_sb, "q")
                    ps_k = proj_pair(wk_sb, "k")
                    qr = [norm_rope(rope, ps_q[ib], ibg0 + ib, "q") for ib in range(2)]
                    ps_v = proj_pair(wv_sb, "v")
                    kr = [norm_rope(rope, ps_k[ib], ibg0 + ib, "k") for ib in range(2)]
                    for ib in range(2):
                        nc.vector.tensor_copy(v_sb[:, ibg0 + ib, :], ps_v[ib][:])
                    for ib in range(2):
                        ibg = ibg0 + ib
                        for h in range(NHL):
                            tp = tp_ps.tile([P, P], F16, name=f"tpq{ibg}_{h}", tag="tp")
                            nc.tensor.transpose(tp[:], qr[ib][:, h * HD:(h + 1) * HD], ident[:])
                            nc.scalar.copy(qt_sb[:, h, ibg * P:(ibg + 1) * P], tp[:])
                        for h in range(NHL):
                            tp = tp_ps.tile([P, P], F16, name=f"tpk{ibg}_{h}", tag="tp")
                            nc.tensor.transpose(tp[:], kr[ib][:, h * HD:(h + 1) * HD], ident[:])
                            nc.scalar.copy(kt_sb[:, h, ibg * P:(ibg + 1) * P], tp[:])

            # ------- Phase B: attention + per-chunk partial o_proj -------
            with ExitStack() as phb:
                s_ps = phb.enter_context(tc.tile_pool(name="s_ps", bufs=4, space="PSUM"))
                yt_psp = phb.enter_context(tc.tile_pool(name="yt_psp", bufs=2, space="PSUM"))
                den_psp = phb.enter_context(tc.tile_pool(name="den_psp", bufs=2, space="PSUM"))

                sps_all = {}

                def emit_score(ic, h, jb):
                    if (ic, h, jb) in sps_all:
                        return
                    off = max(0, P * (jb - 4 * ic))
                    sp = s_ps.tile([P, 512], F32, name=f"s{ic}_{h}_{jb}", tag="s")
                    nc.tensor.matmul(
                        sp[:, off:512],
                        kt_sb[:, h, jb * P:(jb + 1) * P],
                        qt_sb[:, h, ic * 512 + off:(ic + 1) * 512],
                        start=True, stop=True,
                    )
                    sps_all[(ic, h, jb)] = sp

                def emit_finish(ic, h, jb, njb, yt_ps, den_ps):
                    off = max(0, P * (jb - 4 * ic))
                    sp = sps_all.pop((ic, h, jb))
                    et = et_pool.tile([P, 512], F16, name=f"et{ic}_{h}_{jb}", tag="et")
                    nc.scalar.activation(
                        et[:, off:512], sp[:, off:512],
                        mybir.ActivationFunctionType.Exp,
                        bias=neg1_t[:], scale=SCALE,
                    )
                    if jb >= 4 * ic:
                        # diagonal sub-block: zero the upper triangle
                        nc.vector.tensor_mul(
                            et[:, off:off + P], et[:, off:off + P], tri_m[:]
                        )
                    nc.tensor.matmul(
                        yt_ps[:, off:512],
                        v_sb[:, jb, h * HD:(h + 1) * HD],
                        et[:, off:512],
                        start=(jb == 0), stop=(jb == njb - 1),
                    )
                    nc.tensor.matmul(
                        den_ps[:, off:512],
                        ones16[:],
                        et[:, off:512],
                        start=(jb == 0), stop=(jb == njb - 1),
                    )

                for ic in range(ICH):
                    njb = 4 * ic + 4
                    pending_drain = None
                    ytn = {}
                    for h in range(NHL):
                        yt_ps = yt_psp.tile([P, 512], F32, name=f"yt{ic}_{h}", tag="yt")
                        den_ps = den_psp.tile([P, 512], F32, name=f"den{ic}_{h}", tag="den")

                        # three scores of lookahead before the previous head's
                        # drain and before each finish
                        LA = min(3, njb)
                        for jb in range(LA):
                            emit_score(ic, h, jb)
                        if pending_drain is not None:
                            pending_drain()
                            pending_drain = None
                        for jb in range(LA, njb):
                            emit_score(ic, h, jb)
                            emit_finish(ic, h, jb - LA, njb, yt_ps, den_ps)
                        for jb in range(njb - LA, njb):
                            emit_finish(ic, h, jb, njb, yt_ps, den_ps)

                        def make_drain(h=h, ic=ic, yt_ps=yt_ps, den_ps=den_ps):
                            def drain():
                                # den is already replicated across partitions:
                                # one fast approx reciprocal + one multiply
                                rden = bsmall.tile([P, 512], F32, name=f"rd{ic}_{h}", tag="rden")
                                nc.vector.reciprocal_approx_fast(out=rden[:], in_=den_ps[:])
                                yt_sb = ytsb_pool.tile([P, 512], F16, name=f"yts{ic}_{h}", tag="yts")
                                nc.vector.tensor_mul(yt_sb[:], yt_ps[:], rden[:])
                                ytn[h] = yt_sb
                            return drain
                        if h == NHL - 1:
                            # the chunk's o_proj needs ytn[3] promptly
                            make_drain()()
                        else:
                            pending_drain = make_drain()

                    # prime the next chunk's first scores: DMA-independent PE
                    # work that covers the last head's drain chain on the DVE
                    if ic + 1 < ICH:
                        for jb in range(3):
                            emit_score(ic + 1, 0, jb)

                    # ---- partial o_proj for this chunk (local heads only) ----
                    for oc in range(CT):
                        psp = yt_psp if oc % 2 == 0 else den_psp
                        y_ps = psp.tile([P, 512], F32, name=f"zp{ic}_{oc}",
                                        tag="yt" if oc % 2 == 0 else "den")
                        for j in range(NHL):
                            nc.tensor.matmul(
                                y_ps[:],
                                wo_sb[:, j, oc * P:(oc + 1) * P],
                                ytn[j][:],
                                start=(j == 0), stop=(j == NHL - 1),
                            )
                        y_sb = ytsb_pool.tile([P, 512], F16, name=f"zsb{ic}_{oc}", tag="zsb")
                        nc.vector.tensor_copy(y_sb[:], y_ps[:])
                        # sync is a HW-DGE queue and idle during phase B
                        # (gpsimd's queue is software-DGE: too slow here)
                        nc.sync.dma_start(
                            out=z_out[oc, ic, :, :],
                            in_=y_sb[:],
                        )

    nc.compile()
    _program_cache["nc"] = nc
    return nc


def _rope_tables():
    inv_freq = 1.0 / (10000.0 ** (np.arange(0, HD, 2, dtype=np.float32) / HD))
    pos = np.arange(S, dtype=np.float32)
    freqs = np.outer(pos, inv_freq).astype(np.float32)
    cos = np.cos(freqs).astype(np.float16)
    sin = np.sin(freqs).astype(np.float16)
    # pretile [S, HD//2] -> [P, CT, HD//2] with [p, a, f] = t[a*128+p, f]
    cos_t = np.ascontiguousarray(cos.reshape(CT, P, HD // 2).transpose(1, 0, 2))
    sin_t = np.ascontiguousarray(sin.reshape(CT, P, HD // 2).transpose(1, 0, 2))
    return cos_t, sin_t


def make_in_maps(x, Wq, Wk, Wv, Wo):
    x = np.asarray(x, dtype=np.float32)
    cos_t, sin_t = _rope_tables()
    wqT = np.ascontiguousarray(np.asarray(Wq, dtype=np.float32).T.astype(np.float16))
    wkT = np.ascontiguousarray(np.asarray(Wk, dtype=np.float32).T.astype(np.float16))
    wvT = np.ascontiguousarray(np.asarray(Wv, dtype=np.float32).T.astype(np.float16))
    woT = np.ascontiguousarray(np.asarray(Wo, dtype=np.float32).T.astype(np.float16))
    # xT tiled to [ica, ct, p, f] so each (chunk, ct) DMA block is contiguous
    xts = [
        np.ascontiguousarray(
            x[b].T.astype(np.float16)
            .reshape(CT, P, 8, 256).transpose(2, 0, 1, 3)
        )
        for b in range(2)
    ]
    in_maps = []
    for c in range(NCORES):
        b, g = c // 4, c % 4
        sl = slice(g * GW, (g + 1) * GW)
        in_maps.append({
            "xt": xts[b],
            "wq": np.ascontiguousarray(wqT[:, sl]),
            "wk": np.ascontiguousarray(wkT[:, sl]),
            "wv": np.ascontiguousarray(wvT[:, sl]),
            "wo": np.ascontiguousarray(woT[sl, :]),
            "cos": cos_t,
            "sin": sin_t,
        })
    return in_maps


def assemble_output(results):
    y = np.empty((2, S, D), dtype=np.float32)
    for b in range(2):
        zt = np.zeros((CT, ICH, P, 512), dtype=np.float32)
        for g in range(4):
            zt += results[b * 4 + g]["z_out"].astype(np.float32)
        # [oc, ic, p, f] -> z[oc*128+p, ic*512+f] -> y = z.T
        z = zt.transpose(0, 2, 1, 3).reshape(D, S)
        y[b] = z.T
    return y


def kernel(x, Wq, Wk, Wv, Wo):
    nc = build_program()
    in_maps = make_in_maps(x, Wq, Wk, Wv, Wo)
    res = run_bass_kernel_spmd(nc, in_maps, core_ids=list(range(NCORES)))
    return assemble_output(res.results)


# revision 37
# speedup vs baseline: 1.0040x; 1.0003x over previous
"""Causal self-attention (QK-RMSNorm + RoPE) on 8 Trainium2 NeuronCores.

Problem: x[2,2048,2048], Wq/Wk/Wv/Wo [2048,2048], 16 heads, head_dim 128.

Sharding: core c handles batch b=c//4 and head group g=c%4 (4 heads,
model cols [512g:512g+512)).  No collectives: each core computes a
PARTIAL o_proj (contraction over its 4 local head dims only) and
returns zT_partial[2048, 2048]; the host sums the 4 partials per batch
and transposes.  This removes the AllGather tail (17-39us per 512KB
gather, serialized) entirely.

Phase A: Q/K/V projections in ONE pass over host-pre-transposed xT
(contraction dim on partitions), inputs streamed over three parallel
DMA rings.  wo/cos/sin do not compete for startup bandwidth: cos/sin
are host-pretiled to [128,16,64] (contiguous DMA, no SWDGE) and wo is
loaded at the last xt chunk (only needed ~250us in).  ~30 warm-up
matmuls on the identity run during the DMA-dead first ~8us so the HAM
clock gate is warm when the first real matmul issues.
q/k get RMS-norm + RoPE row-wise, are transposed per head on the PE,
and stay resident in SBUF (no DRAM round trip).

Phase B: attention with transposed scores (eT = exp(scale *
kT_blk.T @ qT_chunk)), so the AV matmul (lhsT=v, rhs=eT) directly
yields the transposed output yT[d, i].  Each key block only streams
its causally valid query window (128-granular); a single 128x128
triangular tile masks the diagonal sub-blocks.  Scores are emitted 3
blocks ahead of their exp/AV/den consumers.  The softmax denominator
matmul uses a full ones[128,128] stationary operand so the column
sums come out replicated across all partitions; the drain is then
reciprocal_approx_fast + one multiply on the vector engine.

After each i-chunk's 4 heads are normalized (ytn in SBUF), the
partial o_proj for that chunk runs immediately (64 matmuls:
16 o-tiles x 4 local head contractions) and its z tiles stream out to
DRAM -- no inter-core dependency, so the PE never waits on a
collective.
"""

import math
from contextlib import ExitStack

import numpy as np

import concourse.bass as bass
import concourse.bacc as bacc
import concourse.tile as tile
from concourse import mybir
from concourse.bass_utils import run_bass_kernel_spmd
from concourse.masks import make_identity

P = 128
D = 2048
S = 2048
HD = 128              # head dim
NHL = 4               # heads per core
GW = NHL * HD         # 512, per-core width of head group
CT = D // P           # 16 contraction tiles
ICH = 4               # i-chunks of 512 positions
NCORES = 8
F32 = mybir.dt.float32
F16 = mybir.dt.float16
SCALE = 1.0 / math.sqrt(HD)
EPS = 1.1920928955078125e-07

_program_cache = {}


def build_program():
    if "nc" in _program_cache:
        return _program_cache["nc"]

    nc = bacc.Bacc("TRN2", target_bir_lowering=False, debug=False, num_devices=NCORES)

    # host-pretiled xT: [ica, ct, p, f] = xT[ct*128+p, ica*256+f] so every
    # (chunk, ct) DMA slice is a contiguous 64KB block (strided 512B-segment
    # reads starved the PE during startup)
    xt_in = nc.dram_tensor("xt", [8, CT, P, 256], F16, kind="ExternalInput")
    wq_in = nc.dram_tensor("wq", [D, GW], F16, kind="ExternalInput")
    wk_in = nc.dram_tensor("wk", [D, GW], F16, kind="ExternalInput")
    wv_in = nc.dram_tensor("wv", [D, GW], F16, kind="ExternalInput")
    # Wo.T row slice [512, 2048]: local head dims x full model width
    wo_in = nc.dram_tensor("wo", [GW, D], F16, kind="ExternalInput")
    # host-pretiled rope tables: [p, a, f] = table[a*128+p, f]
    cos_in = nc.dram_tensor("cos", [P, CT, HD // 2], F16, kind="ExternalInput")
    sin_in = nc.dram_tensor("sin", [P, CT, HD // 2], F16, kind="ExternalInput")
    # partial o_proj output, tiled [oc, ic, p, f] = z[oc*128+p, ic*512+f]
    # (contiguous 128KB writes); host sums over 4 cores and untiles
    z_out = nc.dram_tensor("z_out", [CT, ICH, P, 512], F16, kind="ExternalOutput")

    with tile.TileContext(nc) as tc:
        with ExitStack() as ctx:
            const = ctx.enter_context(tc.tile_pool(name="const", bufs=1))

            ident = const.tile([P, P], F16, name="ident")
            make_identity(nc, ident)
            eps_t = const.tile([P, 1], F32, name="eps_t")
            nc.vector.memset(eps_t[:], EPS)
            neg1_t = const.tile([P, 1], F32, name="neg1_t")
            nc.vector.memset(neg1_t[:], -1.0)
            ones_f = const.tile([P, P], F32, name="ones_f")
            nc.vector.memset(ones_f[:], 1.0)
            # full-width ones: den matmul replicates the column sums across
            # all 128 output partitions (same stream cost as 2 partitions)
            ones16 = const.tile([P, P], F16, name="ones16")
            nc.scalar.copy(ones16[:], ones_f[:])
            # tri_m[j, i] = 1 where i >= j (valid), else 0 — diagonal block mask
            tri_m = const.tile([P, P], F16, name="tri_m")
            nc.gpsimd.memset(tri_m[:], 1.0)
            nc.gpsimd.affine_select(
                out=tri_m[:], in_=tri_m[:],
                compare_op=mybir.AluOpType.is_ge,
                fill=0.0,
                base=0,
                pattern=[[1, P]],
                channel_multiplier=-1,
            )

            cos_sb = const.tile([P, CT, HD // 2], F16, name="cos_sb")
            sin_sb = const.tile([P, CT, HD // 2], F16, name="sin_sb")

            # persistent SBUF across phases: transposed q/k, v, and Wo rows
            kv_pool = ctx.enter_context(tc.tile_pool(name="kv_pool", bufs=1))
            qt_sb = kv_pool.tile([P, NHL, S], F16, name="qt_sb")
            kt_sb = kv_pool.tile([P, NHL, S], F16, name="kt_sb")
            v_sb = kv_pool.tile([P, CT, GW], F16, name="v_sb")
            wo_sb = kv_pool.tile([P, NHL, D], F16, name="wo_sb")

            warm_rhs = const.tile([P, 512], F16, name="warm_rhs")
            nc.vector.memset(warm_rhs[:], 0.0)

            # phase B's SBUF pools live alongside phase A's (SBUF has room),
            # so the A->B transition only waits on the PSUM pool swap
            et_pool = ctx.enter_context(tc.tile_pool(name="et_pool", bufs=5))
            bsmall = ctx.enter_context(tc.tile_pool(name="bsmall", bufs=2))
            # all 4 heads' normalized outputs stay live until the chunk's
            # o_proj consumes them
            ytsb_pool = ctx.enter_context(tc.tile_pool(name="ytsb_pool", bufs=5))

            def norm_rope(rope, ps, ibg, t):
                """RMS-norm stats + RoPE on a projection PSUM tile; returns
                the rotated+normalized [P, GW] f16 tile."""
                qs = rope.tile([P, GW], F16, name=f"{t}s{ibg}", tag=f"{t}s")
                nc.scalar.copy(qs[:], ps[:])
                sq = rope.tile([P, GW], F16, name=f"{t}sq{ibg}", tag=f"{t}sq")
                nc.vector.tensor_mul(sq[:], qs[:], qs[:])
                rstd = rope.tile([P, NHL], F32, name=f"{t}rstd{ibg}", tag=f"{t}rstd")
                nc.vector.reduce_sum(
                    rstd[:],
                    sq[:].rearrange("p (h d) -> p h d", h=NHL),
                    axis=mybir.AxisListType.X,
                )
                nc.scalar.activation(
                    rstd[:], rstd[:],
                    mybir.ActivationFunctionType.Sqrt,
                    bias=eps_t[:], scale=1.0 / HD,
                )
                nc.vector.reciprocal(rstd[:], rstd[:])

                q3 = qs[:].rearrange("p (h d) -> p h d", h=NHL)
                qr = rope.tile([P, GW], F16, name=f"{t}r{ibg}", tag=f"{t}r")
                qr3 = qr[:].rearrange("p (h d) -> p h d", h=NHL)
                tmp = rope.tile([P, NHL, HD // 2], F16, name=f"{t}tmp{ibg}", tag=f"{t}tmp")
                cosB = cos_sb[:, ibg:ibg + 1, :].broadcast_to((P, NHL, HD // 2))
                sinB = sin_sb[:, ibg:ibg + 1, :].broadcast_to((P, NHL, HD // 2))
                h1 = q3[:, :, 0:HD // 2]
                h2 = q3[:, :, HD // 2:HD]
                # r1 = q1*cos + q2*sin ; r2 = q2*cos - q1*sin
                nc.vector.tensor_mul(qr3[:, :, 0:HD // 2], h1, cosB)
                nc.vector.tensor_mul(tmp[:], h2, sinB)
                nc.vector.tensor_add(qr3[:, :, 0:HD // 2], qr3[:, :, 0:HD // 2], tmp[:])
                nc.vector.tensor_mul(qr3[:, :, HD // 2:HD], h2, cosB)
                nc.vector.tensor_mul(tmp[:], h1, sinB)
                nc.vector.tensor_sub(
                    qr3[:, :, HD // 2:HD], qr3[:, :, HD // 2:HD], tmp[:]
                )
                for h in range(NHL):
                    nc.vector.tensor_scalar_mul(
                        qr[:, h * HD:(h + 1) * HD],
                        qr[:, h * HD:(h + 1) * HD],
                        rstd[:, h:h + 1],
                    )
                return qr

            # ---------------- Phase A: Q, K, V in one xt pass ----------------
            with ExitStack() as pha:
                wpool = pha.enter_context(tc.tile_pool(name="wpool", bufs=1))
                xt_pool = pha.enter_context(tc.tile_pool(name="xt_pool", bufs=2))
                proj_ps = pha.enter_context(tc.tile_pool(name="proj_ps", bufs=4, space="PSUM"))
                tp_ps = pha.enter_context(tc.tile_pool(name="tp_ps", bufs=2, space="PSUM"))
                rope = pha.enter_context(tc.tile_pool(name="rope", bufs=2))

                wq_sb = wpool.tile([P, CT, GW], F16, name="wq_sb")
                wk_sb = wpool.tile([P, CT, GW], F16, name="wk_sb")
                wv_sb = wpool.tile([P, CT, GW], F16, name="wv_sb")

                # PE warm-up during the DMA-dead startup window: keeps the
                # HAM clock gate warm so the first real matmuls run at 2.4GHz
                wps = proj_ps.tile([P, GW], F32, name="wps", tag="proj")
                for wi in range(12):
                    nc.tensor.matmul(wps[:], ident[:], warm_rhs[:],
                                     start=(wi == 0), stop=(wi == 11))

                # the two HW-DGE queues + scalar share HBM bandwidth roughly
                # per-queue; round-robin every tensor's slices across all
                # three, in global consumption order, so no single stream is
                # capped at 1/3 of the per-core bandwidth during startup
                rr = (nc.sync, nc.gpsimd, nc.scalar)

                def xt_dmas(ica, xt_ch):
                    if ica == 0:
                        for ct in range(CT):
                            rr[ct % 3].dma_start(
                                out=wq_sb[:, ct, :], in_=wq_in[ct * P:(ct + 1) * P, :])
                            rr[(ct + 1) % 3].dma_start(
                                out=xt_ch[:, ct, :], in_=xt_in[ica, ct, :, :])
                        for ct in range(CT):
                            rr[ct % 3].dma_start(
                                out=wk_sb[:, ct, :], in_=wk_in[ct * P:(ct + 1) * P, :])
                        nc.gpsimd.dma_start(out=cos_sb[:], in_=cos_in[:, :, :])
                        nc.sync.dma_start(out=sin_sb[:], in_=sin_in[:, :, :])
                        for ct in range(CT):
                            rr[ct % 3].dma_start(
                                out=wv_sb[:, ct, :], in_=wv_in[ct * P:(ct + 1) * P, :])
                        return
                    for ct in range(CT):
                        eng = rr[(ct + 2) % 3] if ica <= 2 else nc.scalar
                        eng.dma_start(out=xt_ch[:, ct, :], in_=xt_in[ica, ct, :, :])
                    if ica == 6:
                        # wo only needed by the first o_proj (~250us in)
                        for h in range(NHL):
                            nc.gpsimd.dma_start(
                                out=wo_sb[:, h, :],
                                in_=wo_in[h * P:(h + 1) * P, :],
                            )

                for ica in range(8):
                    xt_ch = xt_pool.tile([P, CT, 256], F16, name=f"xt_ch{ica}", tag="xt")
                    xt_dmas(ica, xt_ch)
                    ibg0 = 2 * ica

                    # both ib-halves interleaved per ct: each weight slice
                    # feeds two matmuls back-to-back, halving the startup
                    # DMA-bandwidth demand of the projection stream
                    def proj_pair(wsb, t):
                        pss = []
                        for ib in range(2):
                            pss.append(proj_ps.tile(
                                [P, GW], F32, name=f"ps{t}{ibg0 + ib}", tag="proj"))
                        for ct in range(CT):
                            for ib in range(2):
                                nc.tensor.matmul(
                                    pss[ib][:], xt_ch[:, ct, ib * P:(ib + 1) * P],
                                    wsb[:, ct, :],
                                    start=(ct == 0), stop=(ct == CT - 1),
                                )
                        return pss

                    ps_q = proj_pair(wq_sb, "q")
                    ps_k = proj_pair(wk_sb, "k")
                    qr = [norm_rope(rope, ps_q[ib], ibg0 + ib, "q") for ib in range(2)]
                    ps_v = proj_pair(wv_sb, "v")
                    kr = [norm_rope(rope, ps_k[ib], ibg0 + ib, "k") for ib in range(2)]
                    for ib in range(2):
                        nc.vector.tensor_copy(v_sb[:, ibg0 + ib, :], ps_v[ib][:])
                    for ib in range(2):
                        ibg = ibg0 + ib
                        for h in range(NHL):
                            tp = tp_ps.tile([P, P], F16, name=f"tpq{ibg}_{h}", tag="tp")
                            nc.tensor.transpose(tp[:], qr[ib][:, h * HD:(h + 1) * HD], ident[:])
                            nc.scalar.copy(qt_sb[:, h, ibg * P:(ibg + 1) * P], tp[:])
                        for h in range(NHL):
                            tp = tp_ps.tile([P, P], F16, name=f"tpk{ibg}_{h}", tag="tp")
                            nc.tensor.transpose(tp[:], kr[ib][:, h * HD:(h + 1) * HD], ident[:])
                            nc.scalar.copy(kt_sb[:, h, ibg * P:(ibg + 1) * P], tp[:])

            # ------- Phase B: attention + per-chunk partial o_proj -------
            with ExitStack() as phb:
                s_ps = phb.enter_context(tc.tile_pool(name="s_ps", bufs=4, space="PSUM"))
                yt_psp = phb.enter_context(tc.tile_pool(name="yt_psp", bufs=2, space="PSUM"))
                den_psp = phb.enter_context(tc.tile_pool(name="den_psp", bufs=2, space="PSUM"))

                sps_all = {}

                def emit_score(ic, h, jb):
                    if (ic, h, jb) in sps_all:
                        return
                    off = max(0, P * (jb - 4 * ic))
                    sp = s_ps.tile([P, 512], F32, name=f"s{ic}_{h}_{jb}", tag="s")
                    nc.tensor.matmul(
                        sp[:, off:512],
                        kt_sb[:, h, jb * P:(jb + 1) * P],
                        qt_sb[:, h, ic * 512 + off:(ic + 1) * 512],
                        start=True, stop=True,
                    )
                    sps_all[(ic, h, jb)] = sp

                def emit_finish(ic, h, jb, njb, yt_ps, den_ps):
                    off = max(0, P * (jb - 4 * ic))
                    sp = sps_all.pop((ic, h, jb))
                    et = et_pool.tile([P, 512], F16, name=f"et{ic}_{h}_{jb}", tag="et")
                    nc.scalar.activation(
                        et[:, off:512], sp[:, off:512],
                        mybir.ActivationFunctionType.Exp,
                        bias=neg1_t[:], scale=SCALE,
                    )
                    if jb >= 4 * ic:
                        # diagonal sub-block: zero the upper triangle
                        nc.vector.tensor_mul(
                            et[:, off:off + P], et[:, off:off + P], tri_m[:]
                        )
                    nc.tensor.matmul(
                        yt_ps[:, off:512],
                        v_sb[:, jb, h * HD:(h + 1) * HD],
                        et[:, off:512],
                        start=(jb == 0), stop=(jb == njb - 1),
                    )
                    nc.tensor.matmul(
                        den_ps[:, off:512],
                        ones16[:],
                        et[:, off:512],
                        start=(jb == 0), stop=(jb == njb - 1),
                    )

                for ic in range(ICH):
                    njb = 4 * ic + 4
                    pending_drain = None
                    ytn = {}
                    for h in range(NHL):
                        yt_ps = yt_psp.tile([P, 512], F32, name=f"yt{ic}_{h}", tag="yt")
                        den_ps = den_psp.tile([P, 512], F32, name=f"den{ic}_{h}", tag="den")

                        # three scores of lookahead before the previous head's
                        # drain and before each finish
                        LA = min(3, njb)
                        for jb in range(LA):
                            emit_score(ic, h, jb)
                        if pending_drain is not None:
                            pending_drain()
                            pending_drain = None
                        for jb in range(LA, njb):
                            emit_score(ic, h, jb)
                            emit_finish(ic, h, jb - LA, njb, yt_ps, den_ps)
                        for jb in range(njb - LA, njb):
                            emit_finish(ic, h, jb, njb, yt_ps, den_ps)

                        def make_drain(h=h, ic=ic, yt_ps=yt_ps, den_ps=den_ps):
                            def drain():
                                # den is already replicated across partitions:
                                # one fast approx reciprocal + one multiply
                                rden = bsmall.tile([P, 512], F32, name=f"rd{ic}_{h}", tag="rden")
                                nc.vector.reciprocal_approx_fast(out=rden[:], in_=den_ps[:])
                                yt_sb = ytsb_pool.tile([P, 512], F16, name=f"yts{ic}_{h}", tag="yts")
                                nc.vector.tensor_mul(yt_sb[:], yt_ps[:], rden[:])
                                ytn[h] = yt_sb
                            return drain
                        if h == NHL - 1:
                            # the chunk's o_proj needs ytn[3] promptly
                            make_drain()()
                        else:
                            pending_drain = make_drain()

                    # prime the next chunk's first scores: DMA-independent PE
                    # work that covers the last head's drain chain on the DVE
                    if ic + 1 < ICH:
                        for jb in range(3):
                            emit_score(ic + 1, 0, jb)

                    # ---- partial o_proj for this chunk (local heads only) ----
                    for oc in range(CT):
                        psp = yt_psp if oc % 2 == 0 else den_psp
                        y_ps = psp.tile([P, 512], F32, name=f"zp{ic}_{oc}",
                                        tag="yt" if oc % 2 == 0 else "den")
                        for j in range(NHL):
                            nc.tensor.matmul(
                                y_ps[:],
                                wo_sb[:, j, oc * P:(oc + 1) * P],
                                ytn[j][:],
                                start=(j == 0), stop=(j == NHL - 1),
                            )
                        y_sb = ytsb_pool.tile([P, 512], F16, name=f"zsb{ic}_{oc}",
                                              tag="zsb", bufs=8)
                        nc.vector.tensor_copy(y_sb[:], y_ps[:])
                        # sync is a HW-DGE queue and idle during phase B
                        # (gpsimd's queue is software-DGE: too slow here).
                        # the last chunk's store drain is the exposed tail:
                        # split its TRIGGERS across sync+scalar (scalar's
                        # exp work is over by then) for double drain rate
                        eng = nc.scalar if (ic == ICH - 1 and oc % 2 == 1) else nc.sync
                        eng.dma_start(
                            out=z_out[oc, ic, :, :],
                            in_=y_sb[:],
                        )

    nc.compile()
    _program_cache["nc"] = nc
    return nc


def _rope_tables():
    inv_freq = 1.0 / (10000.0 ** (np.arange(0, HD, 2, dtype=np.float32) / HD))
    pos = np.arange(S, dtype=np.float32)
    freqs = np.outer(pos, inv_freq).astype(np.float32)
    cos = np.cos(freqs).astype(np.float16)
    sin = np.sin(freqs).astype(np.float16)
    # pretile [S, HD//2] -> [P, CT, HD//2] with [p, a, f] = t[a*128+p, f]
    cos_t = np.ascontiguousarray(cos.reshape(CT, P, HD // 2).transpose(1, 0, 2))
    sin_t = np.ascontiguousarray(sin.reshape(CT, P, HD // 2).transpose(1, 0, 2))
    return cos_t, sin_t


def make_in_maps(x, Wq, Wk, Wv, Wo):
    x = np.asarray(x, dtype=np.float32)
    cos_t, sin_t = _rope_tables()
    wqT = np.ascontiguousarray(np.asarray(Wq, dtype=np.float32).T.astype(np.float16))
    wkT = np.ascontiguousarray(np.asarray(Wk, dtype=np.float32).T.astype(np.float16))
    wvT = np.ascontiguousarray(np.asarray(Wv, dtype=np.float32).T.astype(np.float16))
    woT = np.ascontiguousarray(np.asarray(Wo, dtype=np.float32).T.astype(np.float16))
    # xT tiled to [ica, ct, p, f] so each (chunk, ct) DMA block is contiguous
    xts = [
        np.ascontiguousarray(
            x[b].T.astype(np.float16)
            .reshape(CT, P, 8, 256).transpose(2, 0, 1, 3)
        )
        for b in range(2)
    ]
    in_maps = []
    for c in range(NCORES):
        b, g = c // 4, c % 4
        sl = slice(g * GW, (g + 1) * GW)
        in_maps.append({
            "xt": xts[b],
            "wq": np.ascontiguousarray(wqT[:, sl]),
            "wk": np.ascontiguousarray(wkT[:, sl]),
            "wv": np.ascontiguousarray(wvT[:, sl]),
            "wo": np.ascontiguousarray(woT[sl, :]),
            "cos": cos_t,
            "sin": sin_t,
        })
    return in_maps


def assemble_output(results):
    y = np.empty((2, S, D), dtype=np.float32)
    for b in range(2):
        zt = np.zeros((CT, ICH, P, 512), dtype=np.float32)
        for g in range(4):
            zt += results[b * 4 + g]["z_out"].astype(np.float32)
        # [oc, ic, p, f] -> z[oc*128+p, ic*512+f] -> y = z.T
        z = zt.transpose(0, 2, 1, 3).reshape(D, S)
        y[b] = z.T
    return y


def kernel(x, Wq, Wk, Wv, Wo):
    nc = build_program()
    in_maps = make_in_maps(x, Wq, Wk, Wv, Wo)
    res = run_bass_kernel_spmd(nc, in_maps, core_ids=list(range(NCORES)))
    return assemble_output(res.results)
